# revision 5
# baseline (speedup 1.0000x reference)
"""GroupedQueryAttention kernel for 8 Trainium2 NeuronCores.

Shapes (hardcoded): B=2, S=2048, H=2048, NH=16 q heads, NKV=8 kv heads,
HD=128. Sharding: core c owns batch c//4 and GQA groups {2m, 2m+1} where
m = c%4 (q heads 4m..4m+3, kv heads 2m, 2m+1). The host sums the 4
partial o-projections per batch.

Per-core pipeline (bf16 operands, f32 psum accumulation):
  - x is transposed on the HOST (free) and DMA'd as xT tiles; Q^T/K^T
    come straight out of the projection matmul (weights stationary,
    xT moving) so no on-chip transposes at all. V is produced in
    natural [s, hd] layout (xT tile stationary, wv moving).
  - RoPE applied in the transposed [hd, s] domain on DVE via
    host-baked transposed cos/sin tables (sin signed): full-width
    cos multiply, two half-width sin multiplies, full-width add.
  - Flash-style attention with TRANSPOSED score tiles s_T[k, q] =
    KT-tile stationary x QT moving; both q heads of a GQA group are
    packed into [128, 2, 512] tiles (scores psum spans 2 banks, one
    exp / one eacc-add per k-tile). exp (bf16) feeds the PV matmul
    (lhsT = V natural) with zero transposes. Causal k-tiles above
    the diagonal skipped/narrowed; diagonal 128x128 tiles get a
    -1e9 triangular mask. No max-subtraction (scores are O(1)).
    PV matmuls lag one k-tile behind scores/exp (sw pipelining).
  - Softmax denominators: bf16 running sum of exp tiles on DVE
    (eacc), then ONE GpSimd partition_all_reduce per (group, block)
    which both sums over k and broadcasts to all partitions; 1/den
    via reciprocal_approx_fast; normalization fused into the DVE
    psum->sbuf move of the attention output.
  - Fused output projection (stationary = aT chunks, moving = woT),
    two 512-wide output blocks per psum tile -> per-core partial
    o [2048, 2048] bf16, summed on host.
"""

import sys

sys.path.insert(0, "/opt/trn_rl_repo")

import numpy as np

B, S, H = 2, 2048, 2048
NH, NKV, HD = 16, 8, 128
NCORES = 8
QPC = 4                  # q heads per core
KPC = 2                  # kv heads per core
ROPE_BASE = 10000.0
NEG = -1e9

_CACHE = {}


def _rope_tables_T():
    """Transposed rope tables [HD, S] with signed sin (rows 0:63 negated)."""
    inv_freq = 1.0 / (ROPE_BASE ** (np.arange(0, HD, 2, dtype=np.float64) / HD))
    t = np.arange(S, dtype=np.float64)
    freqs = np.outer(t, inv_freq)                       # [S, 64]
    emb = np.concatenate([freqs, freqs], axis=-1)       # [S, 128]
    cos = np.cos(emb).astype(np.float32)
    sin = np.sin(emb).astype(np.float32)
    sin_signed = sin.copy()
    sin_signed[:, : HD // 2] *= -1.0
    return np.ascontiguousarray(cos.T), np.ascontiguousarray(sin_signed.T)


def _build_nc():
    import concourse.bass as bass  # noqa: F401
    import concourse.tile as tile
    from concourse import bacc, bass_isa, mybir
    from concourse.alu_op_type import AluOpType

    f32 = mybir.dt.float32
    bf16 = mybir.dt.bfloat16
    AF = mybir.ActivationFunctionType

    nc = bacc.Bacc("TRN2", target_bir_lowering=False, debug=False)

    NHT = H // 128           # 16 h-tiles (contraction chunks)
    NCK = 4                  # 512-wide s chunks
    NT = S // 128            # 16 s-tiles
    NJ = 4                   # 512-wide q blocks

    xT_d = nc.dram_tensor("xT", [128, NHT, S], bf16, kind="ExternalInput")
    wqkv_d = nc.dram_tensor("wqkvT", [128, NHT, 1024], bf16, kind="ExternalInput")
    woT_d = nc.dram_tensor("woT", [128, QPC, H], bf16, kind="ExternalInput")
    cos_d = nc.dram_tensor("cosT", [HD, S], f32, kind="ExternalInput")
    sin_d = nc.dram_tensor("sinT", [HD, S], f32, kind="ExternalInput")
    tri_d = nc.dram_tensor("tri", [128, 128], f32, kind="ExternalInput")
    o_d = nc.dram_tensor("o_part", [S, H], bf16, kind="ExternalOutput")

    with tile.TileContext(nc) as tc:
        with (
            tc.tile_pool(name="const", bufs=1) as const,
            tc.tile_pool(name="res", bufs=1) as res,
            tc.tile_pool(name="xt", bufs=2) as xtp,
            tc.tile_pool(name="tmp", bufs=3) as tmp,
            tc.tile_pool(name="ep", bufs=3) as epp,
            tc.tile_pool(name="ea", bufs=2) as eap,
            tc.tile_pool(name="dn", bufs=2) as dnp,
            tc.tile_pool(name="rd", bufs=2) as rdp,
            tc.tile_pool(name="oo", bufs=3) as oop,
            tc.tile_pool(name="ps_a", bufs=2, space="PSUM") as ps_a,
            tc.tile_pool(name="ps_b", bufs=2, space="PSUM") as ps_b,
        ):
            # ---- constants; DMA issue order = consumption order ----
            cosT = const.tile([HD, S], f32, tag="cosT")
            nc.sync.dma_start(cosT, cos_d[:, :])
            sinT = const.tile([HD, S], f32, tag="sinT")
            nc.sync.dma_start(sinT, sin_d[:, :])
            xts = []
            for ck in range(NCK):
                xts.append(xtp.tile([128, NHT, 512], bf16, tag="xt",
                                    name=f"xt{ck}"))
            nc.sync.dma_start(xts[0], xT_d[:, :, 0:512])
            wq_t = []
            for ht in range(NHT):
                w = const.tile([128, 1024], bf16, tag=f"wq{ht}")
                nc.sync.dma_start(w, wqkv_d[:, ht, :])
                wq_t.append(w)
            tri = const.tile([128, 128], f32, tag="tri")
            nc.sync.dma_start(tri, tri_d[:, :])
            for ck in range(1, NCK):
                nc.sync.dma_start(xts[ck], xT_d[:, :, ck * 512:(ck + 1) * 512])
            woT = const.tile([128, QPC, H], bf16, tag="woT")
            nc.sync.dma_start(woT, woT_d[:, :, :])

            # ---- per-core resident tensors ----
            QT = res.tile([128, QPC, S], bf16, tag="QT")
            KT = res.tile([128, KPC, S], bf16, tag="KT")
            VN = res.tile([128, NT, KPC * HD], bf16, tag="VN")
            aT = res.tile([128, QPC, S], bf16, tag="aT")

            # ---- projections, per 512-wide s chunk ----
            for ck in range(NCK):
                c0, c1 = ck * 512, (ck + 1) * 512
                xt = xts[ck]
                for st in range(6):          # 4 q + 2 k streams
                    pq = ps_a.tile([128, 2, 512], f32, tag="a",
                                   name=f"pq{st}")
                    for ht in range(NHT):
                        nc.tensor.matmul(
                            pq[:, 0, :],
                            wq_t[ht][:, st * 128:(st + 1) * 128],
                            xt[:, ht, :],
                            start=(ht == 0), stop=(ht == NHT - 1))
                    dst = (QT[:, st, c0:c1] if st < 4
                           else KT[:, st - 4, c0:c1])
                    # RoPE: full cos mult, 2 half sin mults, full add
                    tc_ = tmp.tile([128, 512], f32, tag="tc")
                    nc.vector.tensor_tensor(
                        out=tc_, in0=pq[:, 0, :], in1=cosT[:, c0:c1],
                        op=AluOpType.mult)
                    ts_ = tmp.tile([128, 512], f32, tag="ts")
                    nc.vector.tensor_tensor(
                        out=ts_[0:64, :], in0=pq[64:128, 0, :],
                        in1=sinT[0:64, c0:c1], op=AluOpType.mult)
                    nc.vector.tensor_tensor(
                        out=ts_[64:128, :], in0=pq[0:64, 0, :],
                        in1=sinT[64:128, c0:c1], op=AluOpType.mult)
                    nc.vector.tensor_tensor(
                        out=dst, in0=tc_, in1=ts_, op=AluOpType.add)
                # V natural [s, hd] for both kv heads, per 128-s subtile
                for sv in range(4):
                    pv = ps_b.tile([128, 2, 512], f32, tag="b",
                                   name=f"pv{sv}")
                    for ht in range(NHT):
                        nc.tensor.matmul(
                            pv[:, 0, 0:256],
                            xt[:, ht, sv * 128:(sv + 1) * 128],
                            wq_t[ht][:, 768:1024],
                            start=(ht == 0), stop=(ht == NHT - 1))
                    nc.scalar.copy(VN[:, ck * 4 + sv, :], pv[:, 0, 0:256])

            # ---- attention + fused o-proj, per q-block ----
            for j in range(NJ):
                for g in range(KPC):        # GQA group: q heads 2g, 2g+1
                    ppv = ps_b.tile([128, 2, 512], f32, tag="b",
                                    name=f"ppv{j}{g}")
                    eacc = eap.tile([128, 2, 512], bf16, tag="ea")
                    nkt = 4 * j + 4
                    eps = {}
                    los = {}
                    for kt in range(nkt):
                        m = kt - 4 * j          # >=0 on diagonal tiles
                        lo = max(m, 0) * 128    # first valid q col
                        los[kt] = lo
                        psc = ps_a.tile([128, 2, 512], f32, tag="a",
                                        name="psc")
                        for hh in range(2):
                            nc.tensor.matmul(
                                psc[:, hh, lo:512],
                                KT[:, g, kt * 128:(kt + 1) * 128],
                                QT[:, 2 * g + hh,
                                   j * 512 + lo:(j + 1) * 512],
                                start=True, stop=True)
                            if m >= 0:
                                nc.vector.tensor_tensor(
                                    out=psc[:, hh, lo:lo + 128],
                                    in0=psc[:, hh, lo:lo + 128], in1=tri,
                                    op=AluOpType.add)
                        ep = epp.tile([128, 2, 512], bf16, tag="ep",
                                      name="ep")
                        nc.scalar.activation(
                            ep[:, :, lo:512], psc[:, :, lo:512], AF.Exp)
                        if kt == 0:
                            nc.vector.tensor_copy(eacc, ep)
                        else:
                            nc.vector.tensor_tensor(
                                out=eacc[:, :, lo:512],
                                in0=eacc[:, :, lo:512],
                                in1=ep[:, :, lo:512], op=AluOpType.add)
                        eps[kt] = ep
                        # PV for the PREVIOUS k-tile (sw pipeline)
                        if kt > 0:
                            plo = los[kt - 1]
                            epp_ = eps.pop(kt - 1)
                            for hh in range(2):
                                nc.tensor.matmul(
                                    ppv[:, hh, plo:512],
                                    VN[:, kt - 1, g * 128:(g + 1) * 128],
                                    epp_[:, hh, plo:512],
                                    start=(kt - 1 == 0), stop=False)
                    plo = los[nkt - 1]
                    epp_ = eps.pop(nkt - 1)
                    for hh in range(2):
                        nc.tensor.matmul(
                            ppv[:, hh, plo:512],
                            VN[:, nkt - 1, g * 128:(g + 1) * 128],
                            epp_[:, hh, plo:512],
                            start=(nkt == 1), stop=True)
                    # denominators: sum over k + broadcast in one gpsimd op
                    denb = dnp.tile([128, 2, 512], f32, tag="dn")
                    nc.gpsimd.partition_all_reduce(
                        denb, eacc, 128, bass_isa.ReduceOp.add)
                    rdb = rdp.tile([128, 2, 512], f32, tag="rd")
                    nc.vector.reciprocal_approx_fast(out=rdb, in_=denb)
                    nc.vector.tensor_tensor(
                        out=aT[:, 2 * g:2 * g + 2, j * 512:(j + 1) * 512],
                        in0=ppv, in1=rdb, op=AluOpType.mult)
                # o-proj for this q-block: 4 s-subtiles x 2 hout pairs
                for ss in range(4):
                    r0 = (j * 4 + ss) * 128
                    for hp in range(2):
                        po = ps_b.tile([128, 2, 512], f32, tag="b",
                                       name=f"po{ss}{hp}")
                        for hh in range(2):
                            hb = 2 * hp + hh
                            for t in range(QPC):
                                nc.tensor.matmul(
                                    po[:, hh, :],
                                    aT[:, t, r0:r0 + 128],
                                    woT[:, t, hb * 512:(hb + 1) * 512],
                                    start=(t == 0), stop=(t == QPC - 1))
                        ot = oop.tile([128, 2, 512], bf16, tag="oo")
                        nc.scalar.copy(ot, po)
                        nc.sync.dma_start(
                            o_d[r0:r0 + 128,
                                hp * 1024:(hp + 1) * 1024],
                            ot.rearrange("p a b -> p (a b)"))

    nc.compile()
    return nc


def _get_nc():
    if "nc" not in _CACHE:
        _CACHE["nc"] = _build_nc()
    return _CACHE["nc"]


def _in_maps(hidden_states, wq, wk, wv, wo):
    import ml_dtypes

    bf16 = ml_dtypes.bfloat16
    cosT, sinT = _rope_tables_T()
    tri = np.where(
        np.arange(128)[:, None] <= np.arange(128)[None, :], 0.0, NEG
    ).astype(np.float32)
    scale = 1.0 / np.sqrt(HD)

    NHT = H // 128
    # per-batch xT in [128, NHT, S] layout: xTr[p, ht, s] = x[b, s, ht*128+p]
    xTr = []
    for b in range(B):
        xT = hidden_states[b].astype(np.float32).T          # [H, S]
        xTr.append(np.ascontiguousarray(
            xT.reshape(NHT, 128, S).transpose(1, 0, 2)).astype(bf16))

    maps = []
    for c in range(NCORES):
        b, m = divmod(c, 4)
        wq_c = (wq[m * 4 * HD:(m + 1) * 4 * HD, :] * scale)     # [512, H]
        wk_c = wk[m * 2 * HD:(m + 1) * 2 * HD, :]               # [256, H]
        wv_c = wv[m * 2 * HD:(m + 1) * 2 * HD, :]               # [256, H]
        wqkvT = np.concatenate([wq_c, wk_c, wv_c], axis=0).T    # [H, 1024]
        wqkvTr = np.ascontiguousarray(
            wqkvT.reshape(NHT, 128, 1024).transpose(1, 0, 2)).astype(bf16)
        woT = wo[:, m * 4 * HD:(m + 1) * 4 * HD].T              # [512, H]
        woTr = np.ascontiguousarray(
            woT.reshape(QPC, 128, H).transpose(1, 0, 2)).astype(bf16)
        maps.append({
            "xT": xTr[b], "wqkvT": wqkvTr, "woT": woTr,
            "cosT": cosT, "sinT": sinT, "tri": tri,
        })
    return maps


def run(hidden_states, attention_mask, wq, wk, wv, wo, trace=False):
    from concourse.bass_utils import run_bass_kernel_spmd

    nc = _get_nc()
    maps = _in_maps(hidden_states, wq, wk, wv, wo)
    res = run_bass_kernel_spmd(
        nc, maps, core_ids=list(range(NCORES)), trace=trace)
    out = np.zeros((B, S, H), dtype=np.float32)
    for c, r in enumerate(res.results):
        out[c // 4] += r["o_part"].astype(np.float32)
    return out, res


def kernel(hidden_states, attention_mask, wq, wk, wv, wo):
    out, _ = run(hidden_states, attention_mask, wq, wk, wv, wo, trace=False)
    return out


# revision 11
# speedup vs baseline: 1.2677x; 1.2677x over previous
"""GroupedQueryAttention kernel for 8 Trainium2 NeuronCores.

Shapes (hardcoded): B=2, S=2048, H=2048, NH=16 q heads, NKV=8 kv heads,
HD=128. Sharding: core c owns batch c//4 and GQA groups {2m, 2m+1} where
m = c%4 (q heads 4m..4m+3, kv heads 2m, 2m+1). The host sums the 4
partial o-projections per batch.

Per-core pipeline (bf16 operands, f32 psum accumulation):
  - x is transposed on the HOST (free) and DMA'd as xT tiles; Q^T/K^T
    come straight out of the projection matmul (weights stationary,
    xT moving) so no on-chip transposes at all. V is produced in
    natural [s, hd] layout (xT tile stationary, wv moving).
  - RoPE applied in the transposed [hd, s] domain on DVE via
    host-baked transposed cos/sin tables (sin signed): full-width
    cos multiply, two half-width sin multiplies, full-width add.
  - Flash-style attention with TRANSPOSED score tiles s_T[k, q] =
    KT-tile stationary x QT moving; both q heads of a GQA group are
    packed into [128, 2, 512] tiles (scores psum spans 2 banks, one
    exp / one eacc-add per k-tile). exp (bf16) feeds the PV matmul
    (lhsT = V natural) with zero transposes. Causal k-tiles above
    the diagonal skipped/narrowed; diagonal 128x128 tiles get a
    -1e9 triangular mask. No max-subtraction (scores are O(1)).
    PV matmuls lag one k-tile behind scores/exp (sw pipelining).
  - Softmax denominators: bf16 running sum of exp tiles on DVE
    (eacc), then ONE GpSimd partition_all_reduce per (group, block)
    which both sums over k and broadcasts to all partitions; 1/den
    via reciprocal_approx_fast; normalization fused into the DVE
    psum->sbuf move of the attention output.
  - Fused output projection (stationary = aT chunks, moving = woT),
    two 512-wide output blocks per psum tile -> per-core partial
    o [2048, 2048] bf16, summed on host.
"""

import sys

sys.path.insert(0, "/opt/trn_rl_repo")

import numpy as np

B, S, H = 2, 2048, 2048
NH, NKV, HD = 16, 8, 128
NCORES = 8
QPC = 4                  # q heads per core
KPC = 2                  # kv heads per core
ROPE_BASE = 10000.0
NEG = -1e9

_CACHE = {}


def _rope_tables_T():
    """Transposed rope tables [HD, S] with signed sin (rows 0:63 negated)."""
    inv_freq = 1.0 / (ROPE_BASE ** (np.arange(0, HD, 2, dtype=np.float64) / HD))
    t = np.arange(S, dtype=np.float64)
    freqs = np.outer(t, inv_freq)                       # [S, 64]
    emb = np.concatenate([freqs, freqs], axis=-1)       # [S, 128]
    cos = np.cos(emb).astype(np.float32)
    sin = np.sin(emb).astype(np.float32)
    sin_signed = sin.copy()
    sin_signed[:, : HD // 2] *= -1.0
    return np.ascontiguousarray(cos.T), np.ascontiguousarray(sin_signed.T)


def _build_nc():
    import concourse.bass as bass  # noqa: F401
    import concourse.tile as tile
    from concourse import bacc, bass_isa, mybir
    from concourse.alu_op_type import AluOpType

    f32 = mybir.dt.float32
    bf16 = mybir.dt.bfloat16
    AF = mybir.ActivationFunctionType

    nc = bacc.Bacc("TRN2", target_bir_lowering=False, debug=False)

    NHT = H // 128           # 16 h-tiles (contraction chunks)
    NCK = 4                  # 512-wide s chunks
    NT = S // 128            # 16 s-tiles
    NJ = 4                   # 512-wide q blocks

    xT_d = nc.dram_tensor("xT", [128, NHT, S], bf16, kind="ExternalInput")
    wqkv_d = nc.dram_tensor("wqkvT", [128, NHT, 1024], bf16, kind="ExternalInput")
    woT_d = nc.dram_tensor("woT", [128, QPC, H], bf16, kind="ExternalInput")
    cos_d = nc.dram_tensor("cosT", [HD, S], f32, kind="ExternalInput")
    sin_d = nc.dram_tensor("sinT", [HD, S], f32, kind="ExternalInput")
    tri_d = nc.dram_tensor("tri", [128, 128], f32, kind="ExternalInput")
    onc_d = nc.dram_tensor("ones_col", [128, 1], bf16, kind="ExternalInput")
    onr_d = nc.dram_tensor("ones_row", [1, 128], bf16, kind="ExternalInput")
    o_d = nc.dram_tensor("o_part", [S, H], bf16, kind="ExternalOutput")

    with tile.TileContext(nc) as tc:
        with (
            tc.tile_pool(name="const", bufs=1) as const,
            tc.tile_pool(name="res", bufs=1) as res,
            tc.tile_pool(name="xt", bufs=2) as xtp,
            tc.tile_pool(name="tmp", bufs=3) as tmp,
            tc.tile_pool(name="ep", bufs=3) as epp,
            tc.tile_pool(name="ea", bufs=2) as eap,
            tc.tile_pool(name="dn", bufs=2) as dnp,
            tc.tile_pool(name="rd", bufs=2) as rdp,
            tc.tile_pool(name="oo", bufs=3) as oop,
            tc.tile_pool(name="ps_a", bufs=2, space="PSUM") as ps_a,
            tc.tile_pool(name="ps_b", bufs=2, space="PSUM") as ps_b,
        ):
            # ---- constants; DMA issue order = consumption order ----
            cosT = const.tile([HD, S], f32, tag="cosT")
            nc.sync.dma_start(cosT, cos_d[:, :])
            sinT = const.tile([HD, S], f32, tag="sinT")
            nc.sync.dma_start(sinT, sin_d[:, :])
            xts = []
            for ck in range(NCK):
                xts.append(xtp.tile([128, NHT, 512], bf16, tag="xt",
                                    name=f"xt{ck}"))
            nc.sync.dma_start(xts[0], xT_d[:, :, 0:512])
            wq_t = []
            for ht in range(NHT):
                w = const.tile([128, 1024], bf16, tag=f"wq{ht}")
                nc.sync.dma_start(w, wqkv_d[:, ht, :])
                wq_t.append(w)
            tri = const.tile([128, 128], f32, tag="tri")
            nc.sync.dma_start(tri, tri_d[:, :])
            ones_c = const.tile([128, 1], bf16, tag="ones_c")
            nc.sync.dma_start(ones_c, onc_d[:, :])
            ones_r = const.tile([1, 128], bf16, tag="ones_r")
            nc.sync.dma_start(ones_r, onr_d[:, :])
            for ck in range(1, NCK):
                nc.sync.dma_start(xts[ck], xT_d[:, :, ck * 512:(ck + 1) * 512])
            woT = const.tile([128, QPC, H], bf16, tag="woT")
            nc.sync.dma_start(woT, woT_d[:, :, :])

            # ---- per-core resident tensors ----
            QT = res.tile([128, QPC, S], bf16, tag="QT")
            KT = res.tile([128, KPC, S], bf16, tag="KT")
            VN = res.tile([128, NT, KPC * HD], bf16, tag="VN")
            aT = res.tile([128, QPC, S], bf16, tag="aT")

            # ---- projections, per 512-wide s chunk ----
            for ck in range(NCK):
                c0, c1 = ck * 512, (ck + 1) * 512
                xt = xts[ck]
                for st in range(6):          # 4 q + 2 k streams
                    pq = ps_a.tile([128, 2, 512], f32, tag="a",
                                   name=f"pq{st}")
                    for ht in range(NHT):
                        nc.tensor.matmul(
                            pq[:, 0, :],
                            wq_t[ht][:, st * 128:(st + 1) * 128],
                            xt[:, ht, :],
                            start=(ht == 0), stop=(ht == NHT - 1))
                    dst = (QT[:, st, c0:c1] if st < 4
                           else KT[:, st - 4, c0:c1])
                    # RoPE: full cos mult, 2 half sin mults, full add
                    tc_ = tmp.tile([128, 512], f32, tag="tc")
                    nc.vector.tensor_tensor(
                        out=tc_, in0=pq[:, 0, :], in1=cosT[:, c0:c1],
                        op=AluOpType.mult)
                    ts_ = tmp.tile([128, 512], f32, tag="ts")
                    nc.vector.tensor_tensor(
                        out=ts_[0:64, :], in0=pq[64:128, 0, :],
                        in1=sinT[0:64, c0:c1], op=AluOpType.mult)
                    nc.vector.tensor_tensor(
                        out=ts_[64:128, :], in0=pq[0:64, 0, :],
                        in1=sinT[64:128, c0:c1], op=AluOpType.mult)
                    nc.vector.tensor_tensor(
                        out=dst, in0=tc_, in1=ts_, op=AluOpType.add)
                # V natural [s, hd] for both kv heads, per 128-s subtile
                for sv in range(4):
                    pv = ps_b.tile([128, 2, 512], f32, tag="b",
                                   name=f"pv{sv}")
                    for ht in range(NHT):
                        nc.tensor.matmul(
                            pv[:, 0, 0:256],
                            xt[:, ht, sv * 128:(sv + 1) * 128],
                            wq_t[ht][:, 768:1024],
                            start=(ht == 0), stop=(ht == NHT - 1))
                    nc.scalar.copy(VN[:, ck * 4 + sv, :], pv[:, 0, 0:256])

            # ---- attention + fused o-proj, per q-block ----
            for j in range(NJ):
                for g in range(KPC):        # GQA group: q heads 2g, 2g+1
                    ppv = ps_b.tile([128, 2, 512], f32, tag="b",
                                    name=f"ppv{j}{g}")
                    eacc = eap.tile([128, 2, 512], bf16, tag="ea")
                    nkt = 4 * j + 4
                    eps = {}
                    los = {}
                    for kt in range(nkt):
                        m = kt - 4 * j          # >=0 on diagonal tiles
                        lo = max(m, 0) * 128    # first valid q col
                        los[kt] = lo
                        psc = ps_a.tile([128, 2, 512], f32, tag="a",
                                        name="psc")
                        for hh in range(2):
                            nc.tensor.matmul(
                                psc[:, hh, lo:512],
                                KT[:, g, kt * 128:(kt + 1) * 128],
                                QT[:, 2 * g + hh,
                                   j * 512 + lo:(j + 1) * 512],
                                start=True, stop=True)
                            if m >= 0:
                                nc.vector.tensor_tensor(
                                    out=psc[:, hh, lo:lo + 128],
                                    in0=psc[:, hh, lo:lo + 128], in1=tri,
                                    op=AluOpType.add)
                        ep = epp.tile([128, 2, 512], bf16, tag="ep",
                                      name="ep")
                        nc.scalar.activation(
                            ep[:, :, lo:512], psc[:, :, lo:512], AF.Exp)
                        if kt == 0:
                            nc.vector.tensor_copy(eacc, ep)
                        else:
                            nc.vector.tensor_tensor(
                                out=eacc[:, :, lo:512],
                                in0=eacc[:, :, lo:512],
                                in1=ep[:, :, lo:512], op=AluOpType.add)
                        eps[kt] = ep
                        # PV for the PREVIOUS k-tile (sw pipeline)
                        if kt > 0:
                            plo = los[kt - 1]
                            epp_ = eps.pop(kt - 1)
                            for hh in range(2):
                                nc.tensor.matmul(
                                    ppv[:, hh, plo:512],
                                    VN[:, kt - 1, g * 128:(g + 1) * 128],
                                    epp_[:, hh, plo:512],
                                    start=(kt - 1 == 0), stop=False)
                    plo = los[nkt - 1]
                    epp_ = eps.pop(nkt - 1)
                    for hh in range(2):
                        nc.tensor.matmul(
                            ppv[:, hh, plo:512],
                            VN[:, nkt - 1, g * 128:(g + 1) * 128],
                            epp_[:, hh, plo:512],
                            start=(nkt == 1), stop=True)
                    # denominators: ONE ones-matmul over the sbuf eacc,
                    # then bf16 copy + PE broadcast (ring slots of ps_a)
                    sumt = ps_a.tile([128, 2, 512], f32, tag="a",
                                     name="sumt")
                    for hh in range(2):
                        nc.tensor.matmul(sumt[0:1, hh, :], ones_c,
                                         eacc[:, hh, :],
                                         start=True, stop=True)
                    sum_sb = dnp.tile([1, 2, 512], bf16, tag="dn")
                    nc.scalar.copy(sum_sb, sumt[0:1, :, :])
                    pbc = ps_a.tile([128, 2, 512], f32, tag="a",
                                    name="pbc")
                    for hh in range(2):
                        nc.tensor.matmul(pbc[:, hh, :], ones_r,
                                         sum_sb[:, hh, :],
                                         start=True, stop=True)
                    rdb = rdp.tile([128, 2, 512], f32, tag="rd")
                    nc.vector.reciprocal_approx_fast(out=rdb, in_=pbc)
                    nc.vector.tensor_tensor(
                        out=aT[:, 2 * g:2 * g + 2, j * 512:(j + 1) * 512],
                        in0=ppv, in1=rdb, op=AluOpType.mult)
                # o-proj for this q-block: 4 s-subtiles x 2 hout pairs
                for ss in range(4):
                    r0 = (j * 4 + ss) * 128
                    for hp in range(2):
                        po = ps_b.tile([128, 2, 512], f32, tag="b",
                                       name=f"po{ss}{hp}")
                        for hh in range(2):
                            hb = 2 * hp + hh
                            for t in range(QPC):
                                nc.tensor.matmul(
                                    po[:, hh, :],
                                    aT[:, t, r0:r0 + 128],
                                    woT[:, t, hb * 512:(hb + 1) * 512],
                                    start=(t == 0), stop=(t == QPC - 1))
                        ot = oop.tile([128, 2, 512], bf16, tag="oo")
                        nc.scalar.copy(ot, po)
                        nc.sync.dma_start(
                            o_d[r0:r0 + 128,
                                hp * 1024:(hp + 1) * 1024],
                            ot.rearrange("p a b -> p (a b)"))

    nc.compile()
    return nc


def _get_nc():
    if "nc" not in _CACHE:
        _CACHE["nc"] = _build_nc()
    return _CACHE["nc"]


def _in_maps(hidden_states, wq, wk, wv, wo):
    import ml_dtypes

    bf16 = ml_dtypes.bfloat16
    cosT, sinT = _rope_tables_T()
    tri = np.where(
        np.arange(128)[:, None] <= np.arange(128)[None, :], 0.0, NEG
    ).astype(np.float32)
    ones_col = np.ones((128, 1), bf16)
    ones_row = np.ones((1, 128), bf16)
    scale = 1.0 / np.sqrt(HD)

    NHT = H // 128
    # per-batch xT in [128, NHT, S] layout: xTr[p, ht, s] = x[b, s, ht*128+p]
    xTr = []
    for b in range(B):
        xT = hidden_states[b].astype(np.float32).T          # [H, S]
        xTr.append(np.ascontiguousarray(
            xT.reshape(NHT, 128, S).transpose(1, 0, 2)).astype(bf16))

    maps = []
    for c in range(NCORES):
        b, m = divmod(c, 4)
        wq_c = (wq[m * 4 * HD:(m + 1) * 4 * HD, :] * scale)     # [512, H]
        wk_c = wk[m * 2 * HD:(m + 1) * 2 * HD, :]               # [256, H]
        wv_c = wv[m * 2 * HD:(m + 1) * 2 * HD, :]               # [256, H]
        wqkvT = np.concatenate([wq_c, wk_c, wv_c], axis=0).T    # [H, 1024]
        wqkvTr = np.ascontiguousarray(
            wqkvT.reshape(NHT, 128, 1024).transpose(1, 0, 2)).astype(bf16)
        woT = wo[:, m * 4 * HD:(m + 1) * 4 * HD].T              # [512, H]
        woTr = np.ascontiguousarray(
            woT.reshape(QPC, 128, H).transpose(1, 0, 2)).astype(bf16)
        maps.append({
            "xT": xTr[b], "wqkvT": wqkvTr, "woT": woTr,
            "cosT": cosT, "sinT": sinT, "tri": tri,
            "ones_col": ones_col, "ones_row": ones_row,
        })
    return maps


def run(hidden_states, attention_mask, wq, wk, wv, wo, trace=False):
    from concourse.bass_utils import run_bass_kernel_spmd

    nc = _get_nc()
    maps = _in_maps(hidden_states, wq, wk, wv, wo)
    res = run_bass_kernel_spmd(
        nc, maps, core_ids=list(range(NCORES)), trace=trace)
    out = np.zeros((B, S, H), dtype=np.float32)
    for c, r in enumerate(res.results):
        out[c // 4] += r["o_part"].astype(np.float32)
    return out, res


def kernel(hidden_states, attention_mask, wq, wk, wv, wo):
    out, _ = run(hidden_states, attention_mask, wq, wk, wv, wo, trace=False)
    return out
